# revision 9
# baseline (speedup 1.0000x reference)
"""DeepFM (embedding gather + FM + MLP) Trainium2 kernel.

Strategy: data-parallel over batch across 8 NeuronCores; the 1.7M-row
tables are replicated per core. On device, each core:
  - computes global row ids idx = x + 100000*x_field,
  - gathers 18-wide rows from a combined table
    [emb(16) | lin(1) | ||emb_row||^2(1)] via indirect DMA
    (one descriptor per looked-up row, batch on partitions),
  - scales by x_val (free-dim broadcast), does a log-tree field
    reduction that simultaneously yields the FM first-order sum s,
    sum_f val*lin and sum_f val^2*||v||^2 (the FM sum-of-squares term),
  - PE-transposes the scaled embeddings to feed the 256->64->32->1 MLP
    (BatchNorm folded into the weights on host),
  - combines lin + fm + mlp into a [2,128] PSUM row and DMAs it out.
"""

import os
import sys

for _p in ("/opt/trn_rl_repo", os.path.expanduser("~/.axon_site/_ro/trn_rl_repo")):
    if os.path.isdir(_p) and _p not in sys.path:
        sys.path.insert(0, _p)

import numpy as np

import concourse.bacc as bacc
import concourse.bass as bass
import concourse.mybir as mybir
import concourse.tile as tile
from concourse.bass_utils import run_bass_kernel_spmd

F32 = mybir.dt.float32
I32 = mybir.dt.int32

B = 16384
F = 16
E = 16
TOTAL = 1700000
NCORES = 8
BLOC = B // NCORES          # 2048 batch rows per core
GITER = 256                 # batch rows handled per loop iteration
NITER = BLOC // GITER       # 8
C = 18                      # combined table row: 16 emb + lin + norm2
BN_EPS = 1e-5

_PROGRAM = None


def _build_program():
    """Build (once) the SPMD bass program for one core's 2048-row shard."""
    nc = bacc.Bacc("TRN2", debug=False, target_bir_lowering=False)

    xd = nc.dram_tensor("x_f", [BLOC, F], F32, kind="ExternalInput")
    xfd = nc.dram_tensor("xf_f", [BLOC, F], F32, kind="ExternalInput")
    xvd = nc.dram_tensor("xval", [BLOC, F], F32, kind="ExternalInput")
    tabd = nc.dram_tensor("table18", [TOTAL, C], F32, kind="ExternalInput")
    w1d = nc.dram_tensor("w1f", [2, 128, 64], F32, kind="ExternalInput")
    b1d = nc.dram_tensor("b1f", [64, 1], F32, kind="ExternalInput")
    w2d = nc.dram_tensor("w2f", [64, 32], F32, kind="ExternalInput")
    b2d = nc.dram_tensor("b2f", [32, 1], F32, kind="ExternalInput")
    w3d = nc.dram_tensor("w3f", [32, 1], F32, kind="ExternalInput")
    bias3d = nc.dram_tensor("bias3", [1, 1], F32, kind="ExternalInput")
    identd = nc.dram_tensor("ident", [128, 128], F32, kind="ExternalInput")
    yd = nc.dram_tensor("y", [BLOC], F32, kind="ExternalOutput")

    with tile.TileContext(nc) as tc:
        with (
            tc.tile_pool(name="const", bufs=1) as cp,
            tc.tile_pool(name="gath", bufs=3) as gp,
            tc.tile_pool(name="small", bufs=3) as sp,
            tc.tile_pool(name="mid", bufs=3) as mp,
            tc.tile_pool(name="tpsum", bufs=2, space="PSUM") as tpp,
            tc.tile_pool(name="hpsum", bufs=2, space="PSUM") as hpp,
            tc.tile_pool(name="fpsum", bufs=2, space="PSUM") as fpp,
        ):
            ident = cp.tile([128, 128], F32)
            nc.sync.dma_start(out=ident[:], in_=identd.ap())
            w1s = cp.tile([128, 2 * 64], F32)
            nc.sync.dma_start(
                out=w1s[:].rearrange("p (c n) -> p c n", c=2),
                in_=w1d.ap().rearrange("c p n -> p c n"),
            )
            w2s = cp.tile([64, 32], F32)
            nc.sync.dma_start(out=w2s[:], in_=w2d.ap())
            w3s = cp.tile([32, 1], F32)
            nc.sync.dma_start(out=w3s[:], in_=w3d.ap())
            b1s = cp.tile([64, 1], F32)
            nc.sync.dma_start(out=b1s[:], in_=b1d.ap())
            b2s = cp.tile([32, 1], F32)
            nc.sync.dma_start(out=b2s[:], in_=b2d.ap())
            bias3s = cp.tile([1, 1], F32)
            nc.sync.dma_start(out=bias3s[:], in_=bias3d.ap())

            for it in range(NITER):
                r0 = it * GITER
                # ---- input slices: [128, (g f)] with row b = g*128 + p ----
                src = lambda d: d.ap()[r0 : r0 + GITER, :].rearrange(
                    "(g p) f -> p g f", g=2
                )
                dst3 = lambda t: t[:].rearrange("p (g f) -> p g f", g=2)
                xi = sp.tile([128, 32], F32, name=f"xi{it}", tag="xi")
                nc.sync.dma_start(out=dst3(xi), in_=src(xd))
                xf = sp.tile([128, 32], F32, name=f"xf{it}", tag="xf")
                nc.sync.dma_start(out=dst3(xf), in_=src(xfd))
                xv = sp.tile([128, 32], F32, name=f"xv{it}", tag="xv")
                nc.sync.dma_start(out=dst3(xv), in_=src(xvd))

                # ---- global row ids (int32) ----
                idx = sp.tile([128, 32], I32, name=f"idx{it}", tag="idx")
                nc.vector.scalar_tensor_tensor(
                    out=idx[:],
                    in0=xf[:],
                    scalar=100000.0,
                    in1=xi[:],
                    op0=mybir.AluOpType.mult,
                    op1=mybir.AluOpType.add,
                )

                # ---- gather 18-wide rows: [128, 32*18] ----
                # HW contract: one offset per partition per indirect DMA, so
                # one gather per (group, field) column of 128 rows each.
                g2 = gp.tile([128, 32 * C], F32, name=f"g2{it}", tag="g2")
                for j in range(32):
                    nc.gpsimd.indirect_dma_start(
                        out=g2[:, j * C : (j + 1) * C],
                        out_offset=None,
                        in_=tabd.ap(),
                        in_offset=bass.IndirectOffsetOnAxis(
                            ap=idx[:, j : j + 1], axis=0
                        ),
                    )

                g2v = g2[:].rearrange("p (g f c) -> p g f c", g=2, c=C)
                xvv = xv[:].rearrange("p (g f) -> p g f", g=2)

                # ---- scale by val; emb goes packed [p,(g f e)], lin/norm2
                # ---- to a small tile (norm2 col gets val^2) ----
                es = mp.tile([128, 512], F32, name=f"es{it}", tag="es")
                esv = es[:].rearrange("p (g f e) -> p g f e", g=2, e=16)
                nc.vector.tensor_mul(
                    out=esv,
                    in0=g2v[:, :, :, 0:16],
                    in1=xvv.unsqueeze(3).to_broadcast([128, 2, F, 16]),
                )
                ln = sp.tile([128, 64], F32, name=f"ln{it}", tag="ln")
                lnv = ln[:].rearrange("p (g c2 f) -> p g f c2", g=2, c2=2)
                nc.vector.tensor_mul(
                    out=lnv,
                    in0=g2v[:, :, :, 16:18],
                    in1=xvv.unsqueeze(3).to_broadcast([128, 2, F, 2]),
                )
                ln4 = ln[:].rearrange("p (g c2 f) -> p g c2 f", g=2, c2=2)
                nc.vector.tensor_mul(out=ln4[:, :, 1, :], in0=ln4[:, :, 1, :], in1=xvv)

                # ---- log-tree reduction of scaled emb over fields ----
                t = mp.tile([128, 256], F32, name=f"t{it}", tag="t")
                tv = t[:].rearrange("p (g f e) -> p g f e", g=2, e=16)
                nc.vector.tensor_add(
                    out=tv, in0=esv[:, :, 0:8, :], in1=esv[:, :, 8:16, :]
                )
                nc.vector.tensor_add(
                    out=tv[:, :, 0:4, :], in0=tv[:, :, 0:4, :], in1=tv[:, :, 4:8, :]
                )
                nc.vector.tensor_add(
                    out=tv[:, :, 0:2, :], in0=tv[:, :, 0:2, :], in1=tv[:, :, 2:4, :]
                )
                nc.vector.tensor_add(
                    out=tv[:, :, 0:1, :], in0=tv[:, :, 0:1, :], in1=tv[:, :, 1:2, :]
                )
                red = tv[:, :, 0, :]  # [128, 2, 16] = s

                # ---- FM: 0.5*(sum_e s^2 - sum_f val^2*norm2) + lin ----
                s2 = sp.tile([128, 32], F32, name=f"s2{it}", tag="s2")
                s2v = s2[:].rearrange("p (g e) -> p g e", g=2)
                nc.scalar.square(out=s2v, in_=red)
                acc = sp.tile([128, 2], F32, name=f"acc{it}", tag="acc")
                nc.vector.reduce_sum(out=acc[:], in_=s2v, axis=mybir.AxisListType.X)
                lnsum = sp.tile([128, 4], F32, name=f"lnsum{it}", tag="lnsum")
                lnsumv = lnsum[:].rearrange("p (g c2) -> p g c2", g=2)
                nc.vector.reduce_sum(
                    out=lnsumv,
                    in_=ln[:].rearrange("p (g c2 f) -> p g c2 f", g=2, c2=2),
                    axis=mybir.AxisListType.X,
                )
                nc.vector.tensor_sub(out=acc[:], in0=acc[:], in1=lnsumv[:, :, 1])
                res = sp.tile([128, 2], F32, name=f"res{it}", tag="res")
                nc.vector.scalar_tensor_tensor(
                    out=res[:],
                    in0=acc[:],
                    scalar=0.5,
                    in1=lnsumv[:, :, 0],
                    op0=mybir.AluOpType.mult,
                    op1=mybir.AluOpType.add,
                )

                # ---- MLP: transpose scaled emb to [k, batch] ----
                tr = tpp.tile([128, 512], F32, name=f"tr{it}", tag="tr")
                for c in range(2):
                    for g in range(2):
                        nc.tensor.transpose(
                            out=tr[:, c * 256 + g * 128 : c * 256 + g * 128 + 128],
                            in_=es[:, g * 256 + c * 128 : g * 256 + c * 128 + 128],
                            identity=ident[:],
                        )
                ts = mp.tile([128, 512], F32, name=f"ts{it}", tag="ts")
                nc.vector.tensor_copy(out=ts[:], in_=tr[:])

                h1p = hpp.tile([64, 256], F32, name=f"h1p{it}", tag="h1p")
                for c in range(2):
                    nc.tensor.matmul(
                        out=h1p[:],
                        lhsT=w1s[:].rearrange("p (c n) -> p c n", c=2)[:, c, :],
                        rhs=ts[:, c * 256 : (c + 1) * 256],
                        start=(c == 0),
                        stop=(c == 1),
                    )
                h1 = mp.tile([64, 256], F32, name=f"h1{it}", tag="h1")
                nc.scalar.activation(
                    out=h1[:], in_=h1p[:], func=mybir.ActivationFunctionType.Relu,
                    bias=b1s[:],
                )
                h2p = hpp.tile([32, 256], F32, name=f"h2p{it}", tag="h2p")
                nc.tensor.matmul(out=h2p[:], lhsT=w2s[:], rhs=h1[:])
                h2 = mp.tile([32, 256], F32, name=f"h2{it}", tag="h2")
                nc.scalar.activation(
                    out=h2[:], in_=h2p[:], func=mybir.ActivationFunctionType.Relu,
                    bias=b2s[:],
                )

                # ---- combine mlp + (lin + fm) into [1, 256] ----
                fin = fpp.tile([1, 256], F32, name=f"fin{it}", tag="fin")
                nc.tensor.matmul(
                    out=fin[:],
                    lhsT=w3s[:],
                    rhs=h2[:],
                    start=True,
                    stop=False,
                    skip_group_check=True,
                )
                for g in range(2):
                    nc.tensor.matmul(
                        out=fin[:, g * 128 : (g + 1) * 128],
                        lhsT=res[:, g : g + 1],
                        rhs=ident[:],
                        start=False,
                        stop=(g == 1),
                        skip_group_check=True,
                    )
                yrow = sp.tile([1, 256], F32, name=f"yrow{it}", tag="yrow")
                nc.scalar.activation(
                    out=yrow[:], in_=fin[:],
                    func=mybir.ActivationFunctionType.Identity,
                    bias=bias3s[:],
                )
                nc.sync.dma_start(
                    out=yd.ap()[r0 : r0 + GITER].unsqueeze(0),
                    in_=yrow[:],
                )

    nc.compile()
    return nc


def get_program():
    global _PROGRAM
    if _PROGRAM is None:
        _PROGRAM = _build_program()
    return _PROGRAM


def prepare_host_inputs(inputs):
    """Host-side prep: dtype casts, table packing, BN weight folding."""
    x = np.asarray(inputs["x"])
    x_field = np.asarray(inputs["x_field"])
    x_val = np.asarray(inputs["x_val"], dtype=np.float32)
    emb = np.asarray(inputs["emb_table"], dtype=np.float32)
    lin = np.asarray(inputs["lin_table"], dtype=np.float32)

    table18 = np.empty((TOTAL, C), dtype=np.float32)
    table18[:, 0:16] = emb
    table18[:, 16] = lin[:, 0]
    table18[:, 17] = np.einsum("ij,ij->i", emb, emb)

    gscale1 = np.asarray(inputs["g1"], np.float32) / np.sqrt(np.float32(1.0 + BN_EPS))
    w1f = (np.asarray(inputs["W1"], np.float32) * gscale1[None, :]).reshape(2, 128, 64)
    b1f = (
        np.asarray(inputs["b1"], np.float32) * gscale1
        + np.asarray(inputs["be1"], np.float32)
    ).reshape(64, 1)
    gscale2 = np.asarray(inputs["g2"], np.float32) / np.sqrt(np.float32(1.0 + BN_EPS))
    w2f = np.asarray(inputs["W2"], np.float32) * gscale2[None, :]
    b2f = (
        np.asarray(inputs["b2"], np.float32) * gscale2
        + np.asarray(inputs["be2"], np.float32)
    ).reshape(32, 1)
    w3f = np.asarray(inputs["W3"], np.float32).reshape(32, 1)
    bias3 = np.full(
        (1, 1),
        np.float32(inputs["b3"][0]) + np.float32(inputs["lin_bias"][0]),
        dtype=np.float32,
    )
    ident = np.eye(128, dtype=np.float32)

    shared = {
        "table18": table18,
        "w1f": np.ascontiguousarray(w1f),
        "b1f": b1f,
        "w2f": np.ascontiguousarray(w2f),
        "b2f": b2f,
        "w3f": w3f,
        "bias3": bias3,
        "ident": ident,
    }
    in_maps = []
    for core in range(NCORES):
        sl = slice(core * BLOC, (core + 1) * BLOC)
        in_maps.append(
            {
                "x_f": np.ascontiguousarray(x[sl].astype(np.float32)),
                "xf_f": np.ascontiguousarray(x_field[sl].astype(np.float32)),
                "xval": np.ascontiguousarray(x_val[sl]),
                **shared,
            }
        )
    return in_maps


def run_on_hw(inputs, trace=False):
    nc = get_program()
    in_maps = prepare_host_inputs(inputs)
    r = run_bass_kernel_spmd(nc, in_maps, list(range(NCORES)), trace=trace)
    y = np.concatenate([r.results[i]["y"] for i in range(NCORES)])
    return y, r


def kernel(**inputs):
    y, _ = run_on_hw(inputs)
    return y


def benchmark(inputs, warmup=3, iters=20):
    """Time steady-state NEFF executions on the 8 cores via the PJRT path.

    Returns (y, per_call_ns). Mirrors bass2jax.run_bass_via_pjrt's
    multi-core branch, but builds the jitted callable once and times
    repeated executions with inputs resident on device.
    """
    import time

    import jax
    from jax.experimental.shard_map import shard_map
    from jax.sharding import Mesh, PartitionSpec

    from concourse import bass2jax, mybir as mb

    nc = get_program()
    in_maps = prepare_host_inputs(inputs)
    bass2jax.install_neuronx_cc_hook()

    partition_name = nc.partition_id_tensor.name if nc.partition_id_tensor else None
    in_names, out_names, out_avals, zero_outs = [], [], [], []
    for alloc in nc.m.functions[0].allocations:
        if not isinstance(alloc, mb.MemoryLocationSet):
            continue
        name = alloc.memorylocations[0].name
        if alloc.kind == "ExternalInput":
            if name != partition_name:
                in_names.append(name)
        elif alloc.kind == "ExternalOutput":
            out_names.append(name)
            shape = tuple(alloc.tensor_shape)
            dtype = mb.dt.np(alloc.dtype)
            out_avals.append(jax.core.ShapedArray(shape, dtype))
            zero_outs.append(np.zeros(shape, dtype))
    n_params = len(in_names)
    n_outs = len(out_avals)
    all_in_names = list(in_names) + out_names
    if partition_name is not None:
        all_in_names.append(partition_name)

    def _body(*args):
        operands = list(args)
        if partition_name is not None:
            operands.append(bass2jax.partition_id_tensor())
        outs = bass2jax._bass_exec_p.bind(
            *operands,
            out_avals=tuple(out_avals),
            in_names=tuple(all_in_names),
            out_names=tuple(out_names),
            lowering_input_output_aliases=(),
            sim_require_finite=True,
            sim_require_nnan=True,
            nc=nc,
        )
        return tuple(outs)

    devices = jax.devices()[:NCORES]
    mesh = Mesh(np.asarray(devices), ("core",))
    in_specs = (PartitionSpec("core"),) * (n_params + n_outs)
    out_specs = (PartitionSpec("core"),) * len(out_names)
    sharded = jax.jit(
        shard_map(_body, mesh=mesh, in_specs=in_specs, out_specs=out_specs,
                  check_rep=False),
        donate_argnums=tuple(range(n_params, n_params + n_outs)),
        keep_unused=True,
    )
    per_core = [[np.asarray(m[name]) for name in in_names] for m in in_maps]
    concat_in = [
        np.concatenate([per_core[c][i] for c in range(NCORES)], axis=0)
        for i in range(n_params)
    ]
    from jax.sharding import NamedSharding

    concat_in_dev = [
        jax.device_put(a, NamedSharding(mesh, PartitionSpec("core")))
        for a in concat_in
    ]

    def zeros():
        return [
            np.zeros((NCORES * z.shape[0], *z.shape[1:]), z.dtype) for z in zero_outs
        ]

    out = None
    for _ in range(warmup):
        out = sharded(*concat_in_dev, *zeros())
        jax.block_until_ready(out)

    zs = [zeros() for _ in range(iters)]
    t0 = time.perf_counter()
    outs = []
    for i in range(iters):
        outs.append(sharded(*concat_in_dev, *zs[i]))
    jax.block_until_ready(outs)
    t1 = time.perf_counter()
    per_call_ns = (t1 - t0) / iters * 1e9

    y_all = np.asarray(out[out_names.index("y")]).reshape(NCORES, BLOC)
    y = y_all.reshape(-1)
    return y, per_call_ns


# revision 14
# speedup vs baseline: 27.6668x; 27.6668x over previous
"""DeepFM (embedding gather + FM + MLP) Trainium2 kernel.

Strategy: data-parallel over batch across 8 NeuronCores; the 1.7M-row
tables are replicated per core. On device, each core:
  - computes global row ids idx = x + 100000*x_field,
  - gathers 18-wide rows from a combined table
    [emb(16) | lin(1) | ||emb_row||^2(1)] via indirect DMA
    (one descriptor per looked-up row, batch on partitions),
  - scales by x_val (free-dim broadcast), does a log-tree field
    reduction that simultaneously yields the FM first-order sum s,
    sum_f val*lin and sum_f val^2*||v||^2 (the FM sum-of-squares term),
  - PE-transposes the scaled embeddings to feed the 256->64->32->1 MLP
    (BatchNorm folded into the weights on host),
  - combines lin + fm + mlp into a [2,128] PSUM row and DMAs it out.
"""

import os
import sys

for _p in ("/opt/trn_rl_repo", os.path.expanduser("~/.axon_site/_ro/trn_rl_repo")):
    if os.path.isdir(_p) and _p not in sys.path:
        sys.path.insert(0, _p)

import numpy as np

import concourse.bacc as bacc
import concourse.bass as bass
import concourse.mybir as mybir
import concourse.tile as tile
from concourse.bass_utils import run_bass_kernel_spmd

F32 = mybir.dt.float32
I32 = mybir.dt.int32

B = 16384
F = 16
E = 16
TOTAL = 1700000
NCORES = 8
BLOC = B // NCORES          # 2048 batch rows per core
GITER = 256                 # batch rows handled per loop iteration
NITER = BLOC // GITER       # 8
C = 18                      # combined table row: 16 emb + lin + norm2
BN_EPS = 1e-5

_PROGRAMS = {}


def _build_program(reps=1):
    """Build the SPMD bass program for one core's 2048-row shard.

    reps>1 repeats the whole pipeline (for slope-timing): outputs are
    overwritten each rep, so results are identical.
    """
    nc = bacc.Bacc("TRN2", debug=False, target_bir_lowering=False)

    xd = nc.dram_tensor("x_f", [BLOC, F], F32, kind="ExternalInput")
    xfd = nc.dram_tensor("xf_f", [BLOC, F], F32, kind="ExternalInput")
    xvd = nc.dram_tensor("xval", [BLOC, F], F32, kind="ExternalInput")
    tabd = nc.dram_tensor("table18", [TOTAL, C], F32, kind="ExternalInput")
    w1d = nc.dram_tensor("w1f", [2, 128, 64], F32, kind="ExternalInput")
    b1d = nc.dram_tensor("b1f", [64, 1], F32, kind="ExternalInput")
    w2d = nc.dram_tensor("w2f", [64, 32], F32, kind="ExternalInput")
    b2d = nc.dram_tensor("b2f", [32, 1], F32, kind="ExternalInput")
    w3d = nc.dram_tensor("w3f", [32, 1], F32, kind="ExternalInput")
    bias3d = nc.dram_tensor("bias3", [1, 1], F32, kind="ExternalInput")
    identd = nc.dram_tensor("ident", [128, 128], F32, kind="ExternalInput")
    yd = nc.dram_tensor("y", [BLOC], F32, kind="ExternalOutput")

    with tile.TileContext(nc) as tc:
        with (
            tc.tile_pool(name="const", bufs=1) as cp,
            tc.tile_pool(name="gath", bufs=3) as gp,
            tc.tile_pool(name="small", bufs=3) as sp,
            tc.tile_pool(name="mid", bufs=3) as mp,
            tc.tile_pool(name="tpsum", bufs=2, space="PSUM") as tpp,
            tc.tile_pool(name="hpsum", bufs=2, space="PSUM") as hpp,
            tc.tile_pool(name="fpsum", bufs=2, space="PSUM") as fpp,
        ):
            ident = cp.tile([128, 128], F32)
            nc.sync.dma_start(out=ident[:], in_=identd.ap())
            w1s = cp.tile([128, 2 * 64], F32)
            nc.sync.dma_start(
                out=w1s[:].rearrange("p (c n) -> p c n", c=2),
                in_=w1d.ap().rearrange("c p n -> p c n"),
            )
            w2s = cp.tile([64, 32], F32)
            nc.sync.dma_start(out=w2s[:], in_=w2d.ap())
            w3s = cp.tile([32, 1], F32)
            nc.sync.dma_start(out=w3s[:], in_=w3d.ap())
            b1s = cp.tile([64, 1], F32)
            nc.sync.dma_start(out=b1s[:], in_=b1d.ap())
            b2s = cp.tile([32, 1], F32)
            nc.sync.dma_start(out=b2s[:], in_=b2d.ap())
            bias3s = cp.tile([1, 1], F32)
            nc.sync.dma_start(out=bias3s[:], in_=bias3d.ap())

            for rit in range(reps * NITER):
                it = rit % NITER
                r0 = it * GITER
                # ---- input slices: [128, (g f)] with row b = g*128 + p ----
                src = lambda d: d.ap()[r0 : r0 + GITER, :].rearrange(
                    "(g p) f -> p g f", g=2
                )
                dst3 = lambda t: t[:].rearrange("p (g f) -> p g f", g=2)
                xi = sp.tile([128, 32], F32, name=f"xi{it}", tag="xi")
                nc.sync.dma_start(out=dst3(xi), in_=src(xd))
                xf = sp.tile([128, 32], F32, name=f"xf{it}", tag="xf")
                nc.sync.dma_start(out=dst3(xf), in_=src(xfd))
                xv = sp.tile([128, 32], F32, name=f"xv{it}", tag="xv")
                nc.sync.dma_start(out=dst3(xv), in_=src(xvd))

                # ---- global row ids (int32) ----
                idx = sp.tile([128, 32], I32, name=f"idx{it}", tag="idx")
                nc.vector.scalar_tensor_tensor(
                    out=idx[:],
                    in0=xf[:],
                    scalar=100000.0,
                    in1=xi[:],
                    op0=mybir.AluOpType.mult,
                    op1=mybir.AluOpType.add,
                )

                # ---- gather 18-wide rows: [128, 32*18] ----
                # HW contract: one offset per partition per indirect DMA, so
                # one gather per (group, field) column of 128 rows each.
                g2 = gp.tile([128, 32 * C], F32, name=f"g2{it}", tag="g2")
                for j in range(32):
                    nc.gpsimd.indirect_dma_start(
                        out=g2[:, j * C : (j + 1) * C],
                        out_offset=None,
                        in_=tabd.ap(),
                        in_offset=bass.IndirectOffsetOnAxis(
                            ap=idx[:, j : j + 1], axis=0
                        ),
                    )

                g2v = g2[:].rearrange("p (g f c) -> p g f c", g=2, c=C)
                xvv = xv[:].rearrange("p (g f) -> p g f", g=2)

                # ---- scale by val; emb goes packed [p,(g f e)], lin/norm2
                # ---- to a small tile (norm2 col gets val^2) ----
                es = mp.tile([128, 512], F32, name=f"es{it}", tag="es")
                esv = es[:].rearrange("p (g f e) -> p g f e", g=2, e=16)
                nc.vector.tensor_mul(
                    out=esv,
                    in0=g2v[:, :, :, 0:16],
                    in1=xvv.unsqueeze(3).to_broadcast([128, 2, F, 16]),
                )
                ln = sp.tile([128, 64], F32, name=f"ln{it}", tag="ln")
                lnv = ln[:].rearrange("p (g c2 f) -> p g f c2", g=2, c2=2)
                nc.vector.tensor_mul(
                    out=lnv,
                    in0=g2v[:, :, :, 16:18],
                    in1=xvv.unsqueeze(3).to_broadcast([128, 2, F, 2]),
                )
                ln4 = ln[:].rearrange("p (g c2 f) -> p g c2 f", g=2, c2=2)
                nc.vector.tensor_mul(out=ln4[:, :, 1, :], in0=ln4[:, :, 1, :], in1=xvv)

                # ---- log-tree reduction of scaled emb over fields ----
                t = mp.tile([128, 256], F32, name=f"t{it}", tag="t")
                tv = t[:].rearrange("p (g f e) -> p g f e", g=2, e=16)
                nc.vector.tensor_add(
                    out=tv, in0=esv[:, :, 0:8, :], in1=esv[:, :, 8:16, :]
                )
                nc.vector.tensor_add(
                    out=tv[:, :, 0:4, :], in0=tv[:, :, 0:4, :], in1=tv[:, :, 4:8, :]
                )
                nc.vector.tensor_add(
                    out=tv[:, :, 0:2, :], in0=tv[:, :, 0:2, :], in1=tv[:, :, 2:4, :]
                )
                nc.vector.tensor_add(
                    out=tv[:, :, 0:1, :], in0=tv[:, :, 0:1, :], in1=tv[:, :, 1:2, :]
                )
                red = tv[:, :, 0, :]  # [128, 2, 16] = s

                # ---- FM: 0.5*(sum_e s^2 - sum_f val^2*norm2) + lin ----
                s2 = sp.tile([128, 32], F32, name=f"s2{it}", tag="s2")
                s2v = s2[:].rearrange("p (g e) -> p g e", g=2)
                nc.scalar.square(out=s2v, in_=red)
                acc = sp.tile([128, 2], F32, name=f"acc{it}", tag="acc")
                nc.vector.reduce_sum(out=acc[:], in_=s2v, axis=mybir.AxisListType.X)
                lnsum = sp.tile([128, 4], F32, name=f"lnsum{it}", tag="lnsum")
                lnsumv = lnsum[:].rearrange("p (g c2) -> p g c2", g=2)
                nc.vector.reduce_sum(
                    out=lnsumv,
                    in_=ln[:].rearrange("p (g c2 f) -> p g c2 f", g=2, c2=2),
                    axis=mybir.AxisListType.X,
                )
                nc.vector.tensor_sub(out=acc[:], in0=acc[:], in1=lnsumv[:, :, 1])
                res = sp.tile([128, 2], F32, name=f"res{it}", tag="res")
                nc.vector.scalar_tensor_tensor(
                    out=res[:],
                    in0=acc[:],
                    scalar=0.5,
                    in1=lnsumv[:, :, 0],
                    op0=mybir.AluOpType.mult,
                    op1=mybir.AluOpType.add,
                )

                # ---- MLP: transpose scaled emb to [k, batch] ----
                tr = tpp.tile([128, 512], F32, name=f"tr{it}", tag="tr")
                for c in range(2):
                    for g in range(2):
                        nc.tensor.transpose(
                            out=tr[:, c * 256 + g * 128 : c * 256 + g * 128 + 128],
                            in_=es[:, g * 256 + c * 128 : g * 256 + c * 128 + 128],
                            identity=ident[:],
                        )
                ts = mp.tile([128, 512], F32, name=f"ts{it}", tag="ts")
                nc.vector.tensor_copy(out=ts[:], in_=tr[:])

                h1p = hpp.tile([64, 256], F32, name=f"h1p{it}", tag="h1p")
                for c in range(2):
                    nc.tensor.matmul(
                        out=h1p[:],
                        lhsT=w1s[:].rearrange("p (c n) -> p c n", c=2)[:, c, :],
                        rhs=ts[:, c * 256 : (c + 1) * 256],
                        start=(c == 0),
                        stop=(c == 1),
                    )
                h1 = mp.tile([64, 256], F32, name=f"h1{it}", tag="h1")
                nc.scalar.activation(
                    out=h1[:], in_=h1p[:], func=mybir.ActivationFunctionType.Relu,
                    bias=b1s[:],
                )
                h2p = hpp.tile([32, 256], F32, name=f"h2p{it}", tag="h2p")
                nc.tensor.matmul(out=h2p[:], lhsT=w2s[:], rhs=h1[:])
                h2 = mp.tile([32, 256], F32, name=f"h2{it}", tag="h2")
                nc.scalar.activation(
                    out=h2[:], in_=h2p[:], func=mybir.ActivationFunctionType.Relu,
                    bias=b2s[:],
                )

                # ---- combine mlp + (lin + fm) into [1, 256] ----
                fin = fpp.tile([1, 256], F32, name=f"fin{it}", tag="fin")
                nc.tensor.matmul(
                    out=fin[:],
                    lhsT=w3s[:],
                    rhs=h2[:],
                    start=True,
                    stop=False,
                    skip_group_check=True,
                )
                for g in range(2):
                    nc.tensor.matmul(
                        out=fin[:, g * 128 : (g + 1) * 128],
                        lhsT=res[:, g : g + 1],
                        rhs=ident[:],
                        start=False,
                        stop=(g == 1),
                        skip_group_check=True,
                    )
                yrow = sp.tile([1, 256], F32, name=f"yrow{it}", tag="yrow")
                nc.scalar.activation(
                    out=yrow[:], in_=fin[:],
                    func=mybir.ActivationFunctionType.Identity,
                    bias=bias3s[:],
                )
                nc.sync.dma_start(
                    out=yd.ap()[r0 : r0 + GITER].unsqueeze(0),
                    in_=yrow[:],
                )

    nc.compile()
    return nc


def get_program(reps=1):
    if reps not in _PROGRAMS:
        _PROGRAMS[reps] = _build_program(reps)
    return _PROGRAMS[reps]


def prepare_host_inputs(inputs):
    """Host-side prep: dtype casts, table packing, BN weight folding."""
    x = np.asarray(inputs["x"])
    x_field = np.asarray(inputs["x_field"])
    x_val = np.asarray(inputs["x_val"], dtype=np.float32)
    emb = np.asarray(inputs["emb_table"], dtype=np.float32)
    lin = np.asarray(inputs["lin_table"], dtype=np.float32)

    table18 = np.empty((TOTAL, C), dtype=np.float32)
    table18[:, 0:16] = emb
    table18[:, 16] = lin[:, 0]
    table18[:, 17] = np.einsum("ij,ij->i", emb, emb)

    gscale1 = np.asarray(inputs["g1"], np.float32) / np.sqrt(np.float32(1.0 + BN_EPS))
    w1f = (np.asarray(inputs["W1"], np.float32) * gscale1[None, :]).reshape(2, 128, 64)
    b1f = (
        np.asarray(inputs["b1"], np.float32) * gscale1
        + np.asarray(inputs["be1"], np.float32)
    ).reshape(64, 1)
    gscale2 = np.asarray(inputs["g2"], np.float32) / np.sqrt(np.float32(1.0 + BN_EPS))
    w2f = np.asarray(inputs["W2"], np.float32) * gscale2[None, :]
    b2f = (
        np.asarray(inputs["b2"], np.float32) * gscale2
        + np.asarray(inputs["be2"], np.float32)
    ).reshape(32, 1)
    w3f = np.asarray(inputs["W3"], np.float32).reshape(32, 1)
    bias3 = np.full(
        (1, 1),
        np.float32(inputs["b3"][0]) + np.float32(inputs["lin_bias"][0]),
        dtype=np.float32,
    )
    ident = np.eye(128, dtype=np.float32)

    shared = {
        "table18": table18,
        "w1f": np.ascontiguousarray(w1f),
        "b1f": b1f,
        "w2f": np.ascontiguousarray(w2f),
        "b2f": b2f,
        "w3f": w3f,
        "bias3": bias3,
        "ident": ident,
    }
    in_maps = []
    for core in range(NCORES):
        sl = slice(core * BLOC, (core + 1) * BLOC)
        in_maps.append(
            {
                "x_f": np.ascontiguousarray(x[sl].astype(np.float32)),
                "xf_f": np.ascontiguousarray(x_field[sl].astype(np.float32)),
                "xval": np.ascontiguousarray(x_val[sl]),
                **shared,
            }
        )
    return in_maps


def run_on_hw(inputs, trace=False):
    nc = get_program()
    in_maps = prepare_host_inputs(inputs)
    r = run_bass_kernel_spmd(nc, in_maps, list(range(NCORES)), trace=trace)
    y = np.concatenate([r.results[i]["y"] for i in range(NCORES)])
    return y, r


def kernel(**inputs):
    y, _ = run_on_hw(inputs)
    return y


def make_runner(inputs, reps=1):
    """Build a jitted 8-core runner for the program with given reps.

    Returns (call, out_names) where call() executes once and returns
    the jax output arrays.
    """
    import jax
    from jax.experimental.shard_map import shard_map
    from jax.sharding import Mesh, PartitionSpec

    from concourse import bass2jax, mybir as mb

    nc = get_program(reps)
    in_maps = prepare_host_inputs(inputs)
    bass2jax.install_neuronx_cc_hook()

    partition_name = nc.partition_id_tensor.name if nc.partition_id_tensor else None
    in_names, out_names, out_avals, zero_outs = [], [], [], []
    for alloc in nc.m.functions[0].allocations:
        if not isinstance(alloc, mb.MemoryLocationSet):
            continue
        name = alloc.memorylocations[0].name
        if alloc.kind == "ExternalInput":
            if name != partition_name:
                in_names.append(name)
        elif alloc.kind == "ExternalOutput":
            out_names.append(name)
            shape = tuple(alloc.tensor_shape)
            dtype = mb.dt.np(alloc.dtype)
            out_avals.append(jax.core.ShapedArray(shape, dtype))
            zero_outs.append(np.zeros(shape, dtype))
    n_params = len(in_names)
    n_outs = len(out_avals)
    all_in_names = list(in_names) + out_names
    if partition_name is not None:
        all_in_names.append(partition_name)

    def _body(*args):
        operands = list(args)
        if partition_name is not None:
            operands.append(bass2jax.partition_id_tensor())
        outs = bass2jax._bass_exec_p.bind(
            *operands,
            out_avals=tuple(out_avals),
            in_names=tuple(all_in_names),
            out_names=tuple(out_names),
            lowering_input_output_aliases=(),
            sim_require_finite=True,
            sim_require_nnan=True,
            nc=nc,
        )
        return tuple(outs)

    devices = jax.devices()[:NCORES]
    mesh = Mesh(np.asarray(devices), ("core",))
    in_specs = (PartitionSpec("core"),) * (n_params + n_outs)
    out_specs = (PartitionSpec("core"),) * len(out_names)
    sharded = jax.jit(
        shard_map(_body, mesh=mesh, in_specs=in_specs, out_specs=out_specs,
                  check_rep=False),
        donate_argnums=tuple(range(n_params, n_params + n_outs)),
        keep_unused=True,
    )
    per_core = [[np.asarray(m[name]) for name in in_names] for m in in_maps]
    concat_in = [
        np.concatenate([per_core[c][i] for c in range(NCORES)], axis=0)
        for i in range(n_params)
    ]
    from jax.sharding import NamedSharding

    concat_in_dev = [
        jax.device_put(a, NamedSharding(mesh, PartitionSpec("core")))
        for a in concat_in
    ]

    def zeros():
        return [
            np.zeros((NCORES * z.shape[0], *z.shape[1:]), z.dtype) for z in zero_outs
        ]

    def call():
        return sharded(*concat_in_dev, *zeros())

    return call, out_names


def benchmark(inputs, warmup=3, iters=20, reps=1):
    """Returns (y, mean_per_call_ns, samples_ns) for the reps-program."""
    import time

    import jax

    call, out_names = make_runner(inputs, reps)
    out = None
    for _ in range(warmup):
        out = call()
        jax.block_until_ready(out)
    samples = []
    for _ in range(iters):
        t0 = time.perf_counter()
        o = call()
        jax.block_until_ready(o)
        samples.append((time.perf_counter() - t0) * 1e9)
    y = np.asarray(out[out_names.index("y")]).reshape(-1)
    return y, float(np.mean(samples)), samples
